# revision 35
# baseline (speedup 1.0000x reference)
"""Deformable conv (DCNv2-style) TRN2 Bass kernel — DMA-restructured.

Problem: x[8,64,128,128] f32; offset conv (27ch 3x3) -> (dy,dx,mask) per 9 taps;
bilinear sampling of x at tap positions + offsets; modulated; 3x3 conv via
per-tap 1x1 matmuls.  Data-parallel over batch, 1 sample per core.

Per-core pipeline:
  A) om conv: 9 shifted f32r matmuls per 512-pos chunk on zero-padded XP
     -> OM_ALL [27, 16384]; ONE transpose DMA -> OMT [h, j, w].
  B) stage2 (DVE/ACT on [128h, 9t, 128w]): floor/clip/hat-weights/sigmoid
     -> W4C (bf16 corner-weight pairs) and IDXF (u16 element idx into X3).
  C) X3: interleaved row-pair layout X3[c, 2*(128r+j)+s] = x[c, r+s, j] (bf16);
     partitions 64-127 hold X3 shifted by 2 elements (x0+1 corners).
  D) IDXT: 16+3 DMAs -> idx wrapped per 16 partitions [p, t, m, col].
     WLIN staging: W4C -> DRAM as [t][half, m, hl, w, slot] (18 small DMAs).
  E) main loop over 4 Qs x 9 taps: ONE indirect_copy (4096 idx, d=2),
     ONE 2 MB 128-partition broadcast DMA for weights, ONE DVE mult,
     16 accumulating matmuls -> PSUM [64,512] x 8 chunks; per Q: 8 ACT
     copies -> OSBQ, ONE 1 MB DMA to out.

Gather stream order: position s = 512*m + col*16 + p.
"""
import numpy as np
import ml_dtypes

from concourse.bacc import Bacc
from concourse import mybir, tile
from concourse.bass_utils import run_bass_kernel_spmd

np_bf16 = np.dtype(ml_dtypes.bfloat16)
f32 = mybir.dt.float32
f32r = mybir.dt.float32r
bf16 = mybir.dt.bfloat16
u16 = mybir.dt.uint16
i32 = mybir.dt.int32

B, C, H, W = 8, 64, 128, 128
HW = H * W          # 16384
T = 9               # taps
NJ = 27             # offset-conv channels
NCHUNK = 32         # 512-position chunks
CHUNK = 512
NQ = 4              # chunk groups
QC = NCHUNK // NQ   # 8 chunks per group
AF = mybir.ActivationFunctionType
ALU = mybir.AluOpType

_CACHE = {}


def _host_consts():
    # CYK[h, t] = h + ky(t) - 1 ; CXW[h, t, w] = w + kx(t) - 1 (h-independent)
    ky = np.arange(T) // 3
    kx = np.arange(T) % 3
    cyk = (np.arange(128)[:, None] + ky[None, :] - 1).astype(np.float32)
    cxw = np.broadcast_to(
        (np.arange(128)[None, :] + kx[:, None] - 1)[None, :, :], (128, T, 128)
    ).astype(np.float32).copy()
    return cyk, cxw


def build_nc(num_devices=8):
    import os
    variant = os.environ.get("KVARIANT", "full")
    v_wrq2 = "wrq2" in variant or "oldall" in variant
    v_omtold = "omtold" in variant or "oldall" in variant
    v_idxold = "idxold" in variant or "oldall" in variant
    nc = Bacc("TRN2", target_bir_lowering=False, debug=False,
              num_devices=num_devices)
    kdebug = os.environ.get("KDEBUG") == "1"

    x_in = nc.dram_tensor("x_in", [C, HW], f32, kind="ExternalInput")
    woffT_in = nc.dram_tensor("woffT_in", [C, T * NJ], f32, kind="ExternalInput")
    boff_in = nc.dram_tensor("boff_in", [NJ, 1], f32, kind="ExternalInput")
    wk2_in = nc.dram_tensor("wk2_in", [128, T * C], bf16, kind="ExternalInput")
    out_dram = nc.dram_tensor("out", [C, HW], f32, kind="ExternalOutput")

    cyk_np, cxw_np = _host_consts()
    cyk_const = nc.inline_tensor(cyk_np, name="cyk_const")
    cxw_const = nc.inline_tensor(cxw_np.reshape(128, T * 128), name="cxw_const")

    with tile.TileContext(nc) as tc:
        with tc.tile_pool(name="main", bufs=1) as mp, \
             tc.tile_pool(name="dram", bufs=1, space="DRAM") as drp:
            # ---------- persistent tiles ----------
            X3 = mp.tile([128, HW * 2 + 16], bf16, tag="X3")     # 64 KiB + pad
            W4C = mp.tile([128, 2, T, 128, 2], bf16, tag="W4C")  # [h,half,t,w,slot]
            IDXF = mp.tile([128, T, 8, 16], u16, tag="IDXF")     # [h,t,a,b] (a=w//16,b=w%16)
            IDXT = mp.tile([128, T, NCHUNK, 32], u16, tag="IDXT")
            # WLIN row (t*2+half) = [m, hl, w, slot] flat; DRAM staging for
            # broadcast reads (flat 2-D: >2-D DRAM tile slicing miscompiles)
            WLIN = drp.tile([T * 2, NCHUNK * 4 * 128 * 2], bf16, tag="WLIN")
            CYK = mp.tile([128, T], f32, tag="CYK")
            CXW = mp.tile([128, T, 128], f32, tag="CXW")
            WOFFT = mp.tile([C, T, NJ], bf16, tag="WOFFT")
            WK2 = mp.tile([128, T, C], bf16, tag="WK2")
            BOFF = mp.tile([NJ, 1], f32, tag="BOFF")

            nc.sync.dma_start(CYK[:], cyk_const.ap())
            nc.sync.dma_start(CXW[:].rearrange("p a b -> p (a b)"), cxw_const.ap())
            nc.sync.dma_start(WK2[:].rearrange("p a b -> p (a b)"), wk2_in.ap())
            nc.sync.dma_start(BOFF[:], boff_in.ap())

            # ================= Phase A: pad + X3 + om conv =================
            midcm = tc.tile_pool(name="mid", bufs=1)
            midp = midcm.__enter__()
            OMT = midp.tile([128, NJ, 128], f32, tag="OMT")      # [h, j, w]
            with tc.tile_pool(name="early", bufs=1) as ep, \
                 tc.tile_pool(name="omdb", bufs=2) as odb, \
                 tc.tile_pool(name="ompsum", bufs=2, space="PSUM") as opp:
                XP = ep.tile([C, 130 * 130], bf16, tag="XP")
                WOFFS = ep.tile([C, T * NJ], f32, tag="WOFFS")
                nc.sync.dma_start(WOFFS[:], woffT_in.ap())
                nc.vector.tensor_copy(out=WOFFT[:].rearrange("p a b -> p (a b)"),
                                      in_=WOFFS[:])

                # zero only the halo ring of XP (interior is overwritten by x)
                XP3 = XP[:].rearrange("p (r c2) -> p r c2", c2=130)
                nc.vector.memset(XP[:, 0:130], 0.0)                  # top row
                nc.vector.memset(XP[:, 129 * 130:130 * 130], 0.0)    # bottom row
                nc.vector.memset(XP3[:, 1:129, 0], 0.0)              # left col
                nc.vector.memset(XP3[:, 1:129, 129], 0.0)            # right col
                nc.gpsimd.dma_start(out=XP3[:, 1:129, 1:129], in_=x_in.ap())

                # X3 A-half: X3[c, r*256 + 2j + s] = x[c, r+s, j]
                X3A = X3[0:64, 0:HW * 2].rearrange("p (r j s) -> p r j s", j=128, s=2)
                nc.scalar.copy(out=X3A[:, :, :, 0], in_=XP3[0:64, 1:129, 1:129])
                nc.vector.tensor_copy(out=X3A[:, :, :, 1], in_=XP3[0:64, 2:130, 1:129])
                nc.vector.memset(X3[0:64, HW * 2:], 0.0)
                # B-half: shift by 2 elements (cross-partition copy via DMA)
                nc.sync.dma_start(out=X3[64:128, 0:2 * HW - 2], in_=X3[0:64, 2:2 * HW])
                nc.vector.memset(X3[64:128, 2 * HW - 2:], 0.0)

                # om conv: per 512-pos chunk, 9 accumulating f32r matmuls;
                # stage 4 chunks into omp4 -> DRAM scratch -> transpose load
                OMD = drp.tile([NJ, HW], f32, tag="OMD")   # DRAM [j, (h w)]
                for g in range(NCHUNK // 4):
                    omp4 = odb.tile([NJ, 4, CHUNK], f32, tag="omp4")
                    for mi in range(4):
                        m = 4 * g + mi
                        ps = opp.tile([NJ, CHUNK], f32, tag="omps")
                        for t9 in range(T):
                            ty, tx = divmod(t9, 3)
                            rhs = XP3[0:64, 4 * m + ty: 4 * m + ty + 4, tx: tx + 128]
                            nc.tensor.matmul(ps[:], lhsT=WOFFT[:, t9, :], rhs=rhs,
                                             start=(t9 == 0), stop=(t9 == T - 1))
                        nc.scalar.activation(out=omp4[:, mi, :], in_=ps[:],
                                             func=AF.Identity, bias=BOFF[:])
                    if v_omtold:
                        ompv = omp4[:].rearrange("j m4 (hp w) -> j (m4 hp) w", w=128)
                        for hp in range(16):
                            nc.sync.dma_start(
                                out=OMT[16 * g + hp:16 * g + hp + 1, :, :],
                                in_=ompv[:, hp, :])
                    else:
                        nc.sync.dma_start(
                            out=OMD[:][:, 2048 * g:2048 * (g + 1)],
                            in_=omp4[:].rearrange("j m4 p -> j (m4 p)"))
                if not v_omtold:
                    # ONE transpose load: OMT[h, j, w] <- OMD[j, (h w)]
                    nc.sync.dma_start(
                        out=OMT[:],
                        in_=OMD[:].rearrange("j (h w) -> h j w", w=128))
            # ================= stage 2: weights + idx =================
            with tc.tile_pool(name="s2", bufs=1) as sp:
                OMTv = OMT[:]
                DY = OMTv[:, 0:18, :].rearrange("p (k s) w -> p k s w", s=2)[:, :, 0, :]
                DX = OMTv[:, 0:18, :].rearrange("p (k s) w -> p k s w", s=2)[:, :, 1, :]
                MS = OMTv[:, 18:27, :]

                sh = [128, T, 128]
                YS = sp.tile(sh, f32, tag="YS")
                XS = sp.tile(sh, f32, tag="XS")
                Y0C = sp.tile(sh, f32, tag="Y0C")
                X0C = sp.tile(sh, f32, tag="X0C")
                TMPI = sp.tile(sh, i32, tag="TMPI")
                TY = sp.tile(sh, f32, tag="TY")
                TX = sp.tile(sh, f32, tag="TX")
                WYA = sp.tile(sh, f32, tag="WYA")
                WYB = sp.tile(sh, f32, tag="WYB")
                WXA = sp.tile(sh, f32, tag="WXA")
                WXB = sp.tile(sh, f32, tag="WXB")
                MSK = sp.tile(sh, f32, tag="MSK")
                TMP = sp.tile(sh, f32, tag="TMP")
                TMP2 = sp.tile(sh, f32, tag="TMP2")

                CYKb = CYK[:].unsqueeze(2).broadcast_to(sh)

                # ys/xs
                nc.vector.tensor_tensor(out=YS[:], in0=DY, in1=CYKb, op=ALU.add)
                nc.vector.tensor_tensor(out=XS[:], in0=DX, in1=CXW[:], op=ALU.add)
                # floor via round(x - 0.5) (cast round-to-nearest-even)
                for SRC, DSTF in ((YS, Y0C), (XS, X0C)):
                    nc.vector.tensor_scalar(out=TMP[:], in0=SRC[:], scalar1=0.5,
                                            scalar2=None, op0=ALU.subtract)
                    nc.vector.tensor_copy(out=TMPI[:], in_=TMP[:])
                    nc.vector.tensor_copy(out=DSTF[:], in_=TMPI[:])
                    # clip to [0, 127]
                    nc.vector.tensor_scalar(out=DSTF[:], in0=DSTF[:], scalar1=0.0,
                                            scalar2=127.0, op0=ALU.max, op1=ALU.min)
                # t = s - clip ; weights
                nc.vector.tensor_tensor(out=TY[:], in0=YS[:], in1=Y0C[:], op=ALU.subtract)
                nc.vector.tensor_tensor(out=TX[:], in0=XS[:], in1=X0C[:], op=ALU.subtract)
                # wA = relu(1 - |t|), wBr = relu(t)
                nc.scalar.activation(out=TMP[:], in_=TY[:], func=AF.Abs)
                nc.scalar.activation(out=WYA[:], in_=TMP[:], func=AF.Relu, scale=-1.0, bias=1.0)
                nc.scalar.activation(out=WYB[:], in_=TY[:], func=AF.Relu)
                nc.scalar.activation(out=TMP2[:], in_=TX[:], func=AF.Abs)
                nc.scalar.activation(out=WXA[:], in_=TMP2[:], func=AF.Relu, scale=-1.0, bias=1.0)
                nc.scalar.activation(out=WXB[:], in_=TX[:], func=AF.Relu)
                # upper-boundary masks: wyB *= (ys < 127); wxB *= (xs < 127)
                nc.vector.tensor_scalar(out=TMP[:], in0=YS[:], scalar1=127.0,
                                        scalar2=None, op0=ALU.is_lt)
                nc.vector.tensor_tensor(out=WYB[:], in0=WYB[:], in1=TMP[:], op=ALU.mult)
                nc.vector.tensor_scalar(out=TMP2[:], in0=XS[:], scalar1=127.0,
                                        scalar2=None, op0=ALU.is_lt)
                nc.vector.tensor_tensor(out=WXB[:], in0=WXB[:], in1=TMP2[:], op=ALU.mult)
                # mask; fold into wx
                nc.scalar.activation(out=MSK[:], in_=MS, func=AF.Sigmoid)
                nc.vector.tensor_tensor(out=WXA[:], in0=WXA[:], in1=MSK[:], op=ALU.mult)
                nc.vector.tensor_tensor(out=WXB[:], in0=WXB[:], in1=MSK[:], op=ALU.mult)
                # products -> W4C (bf16, interleaved)
                nc.vector.tensor_tensor(out=W4C[:, 0, :, :, 0], in0=WYA[:], in1=WXA[:], op=ALU.mult)
                nc.vector.tensor_tensor(out=W4C[:, 0, :, :, 1], in0=WYB[:], in1=WXA[:], op=ALU.mult)
                nc.vector.tensor_tensor(out=W4C[:, 1, :, :, 0], in0=WYA[:], in1=WXB[:], op=ALU.mult)
                nc.vector.tensor_tensor(out=W4C[:, 1, :, :, 1], in0=WYB[:], in1=WXB[:], op=ALU.mult)
                # idx (element units into X3): 2*(y0c*128 + x0c), written in
                # IDXF layout [h, t, a, b] with value for w = 16a + b
                nc.vector.scalar_tensor_tensor(
                    out=TMP[:], in0=Y0C[:], scalar=128.0, in1=X0C[:],
                    op0=ALU.mult, op1=ALU.add)
                IDXF_w = IDXF[:].rearrange("p t a b2 -> p (t a b2)").rearrange(
                    "p (t a b2) -> p t a b2", t=T, a=8)
                nc.vector.tensor_scalar(
                    out=IDXF_w, in0=TMP[:].rearrange("p t (a b2) -> p t a b2", a=8),
                    scalar1=2.0, scalar2=None, op0=ALU.mult)

            # ---------- IDXT build: DRAM bounce + 16 loads + 3 replications ----
            # IDXT[p, t, m, col] = idx(s = 512m + col*16 + p)
            #                    = IDXF[h = 4m + col//8, t, a = col%8, b = p]
            if v_idxold:
                for p16 in range(16):
                    for t9 in range(T):
                        nc.sync.dma_start(
                            out=IDXT[p16:p16 + 1, t9, :, :].rearrange(
                                "o m (hl a) -> o (m hl) a", hl=4),
                            in_=IDXF[:, t9, :, p16])
            else:
                # hardware-xbar transpose: ITF[(a b), t, h] = IDXF[h, t, a, b]
                ITF = midp.tile([128, T, 128], u16, tag="ITF")
                for t9 in range(T):
                    nc.sync.dma_start_transpose(
                        out=ITF[:, t9, :],
                        in_=IDXF[:, t9, :, :].rearrange("p a b -> p (a b)"))
                # bounce through DRAM, then ONE load:
                # IDXT[p, t, m, (hl a)] = ITF[16a+p, t, (m hl)]
                IDXD = drp.tile([128, T * 128], u16, tag="IDXD")
                nc.sync.dma_start(out=IDXD[:],
                                  in_=ITF[:].rearrange("p t h -> p (t h)"))
                for a8 in range(8):
                    nc.sync.dma_start(
                        out=IDXT[0:16, :, :, a8::8].rearrange(
                            "p t m hl -> p (t m hl)"),
                        in_=IDXD[16 * a8:16 * (a8 + 1), :])
            # replicate partitions 0-15 -> 16-127 (doubling)
            nc.sync.dma_start(out=IDXT[16:32], in_=IDXT[0:16])
            nc.sync.dma_start(out=IDXT[32:64], in_=IDXT[0:32])
            nc.sync.dma_start(out=IDXT[64:128], in_=IDXT[0:64])
            midcm.__exit__(None, None, None)

            if kdebug:
                d_omt = nc.dram_tensor("d_omt", [128, NJ * 128], f32, kind="ExternalOutput")
                nc.sync.dma_start(d_omt.ap(), OMT[:].rearrange("p a b -> p (a b)"))
                d_w4 = nc.dram_tensor("d_w4", [128, 2 * T * 128 * 2], bf16, kind="ExternalOutput")
                nc.sync.dma_start(d_w4.ap(), W4C[:].rearrange("p a b c d -> p (a b c d)"))
                d_idxt = nc.dram_tensor("d_idxt", [128, T * NCHUNK * 32], u16, kind="ExternalOutput")
                nc.sync.dma_start(d_idxt.ap(), IDXT[:].rearrange("p a b c -> p (a b c)"))
                d_x3 = nc.dram_tensor("d_x3", [128, HW * 2], bf16, kind="ExternalOutput")
                nc.sync.dma_start(d_x3.ap(), X3[:, 0:HW * 2])

            # ---------- WLIN build: 18 small DMAs (SBUF -> DRAM) ----------
            # WLIN[t*2+half] = flat [m, hl, w, slot] = W4C[h=(m hl), half, t, w, s]
            WLINv = WLIN[:]
            for t9 in range(T):
                for half in range(2):
                    nc.sync.dma_start(
                        out=WLINv[t9 * 2 + half].rearrange("(h f) -> h f", h=128),
                        in_=W4C[:, half, t9, :, :].rearrange("p a b -> p (a b)"))

            if kdebug:
                d_wlin = nc.dram_tensor("d_wlin", [T * 2, NCHUNK * 4 * 128 * 2], bf16, kind="ExternalOutput")
                nc.sync.dma_start(d_wlin.ap(), WLINv)

            # ================= main loop =================
            with tc.tile_pool(name="gl", bufs=2) as gp, \
                 tc.tile_pool(name="ps2", bufs=1, space="PSUM") as pp2:
                X3v = X3[:, 0:HW * 2].rearrange("p (n s) -> p n s", s=2)
                # absorb initial deps into gpsimd queue
                dd1 = mp.tile([128, 1], bf16, tag="dd1")
                dd2 = mp.tile([128, 1], u16, tag="dd2")
                nc.gpsimd.tensor_copy(out=dd1[:], in_=X3[:, 0:1])
                nc.gpsimd.tensor_copy(out=dd2[:], in_=IDXT[:, 0, 0, 0:1])

                for Q in range(NQ):
                    psums = [pp2.tile([C, CHUNK], f32, tag=f"eps{c8}",
                                      name=f"eps_{Q}_{c8}")
                             for c8 in range(QC)]
                    for tap in range(T):
                        GQ = gp.tile([128, QC * CHUNK, 2], bf16, tag="GQ")
                        WRQ = gp.tile([128, QC, CHUNK, 2], bf16, tag="WRQ")
                        # gathers: per chunk (IC dst cap = 1024 elems/partition)
                        for c8 in range(QC):
                            nc.gpsimd.indirect_copy(
                                out=GQ[:, CHUNK * c8:CHUNK * (c8 + 1), :],
                                data=X3v,
                                idxs=IDXT[:, tap, QC * Q + c8, :],
                                i_know_ap_gather_is_preferred=True)
                        # 128-partition broadcast DMA (2 MB):
                        # WRQ[(half c), m, pos, slot] <- WLIN[tap, half, m(Q), :, :, :]
                        QB = QC * CHUNK * 2   # elems per (tap, half, Q)
                        for half in range(2):
                            eng = [nc.scalar, nc.sync,
                                   nc.gpsimd][(2 * tap + half) % 3]
                            eng.dma_start(
                                out=WRQ[64 * half:64 * (half + 1)],
                                in_=WLINv[tap * 2 + half,
                                          QB * Q:QB * (Q + 1)]
                                    .unsqueeze(0)
                                    .broadcast_to([64, QB])
                                    .rearrange("c (m pos s) -> c m pos s",
                                               m=QC, s=2))
                        if kdebug and Q == 0 and tap == 0:
                            d_gq = nc.dram_tensor("d_gq", [128, QC * CHUNK * 2], bf16, kind="ExternalOutput")
                            nc.sync.dma_start(d_gq.ap(), GQ[:].rearrange("p a b -> p (a b)"))
                            d_wrq = nc.dram_tensor("d_wrq", [128, QC * CHUNK * 2], bf16, kind="ExternalOutput")
                            nc.sync.dma_start(d_wrq.ap(), WRQ[:].rearrange("p a b c -> p (a b c)"))
                        nc.vector.tensor_tensor(
                            out=GQ[:].rearrange("p a b -> p (a b)"),
                            in0=GQ[:].rearrange("p a b -> p (a b)"),
                            in1=WRQ[:].rearrange("p a b c -> p (a b c)"),
                            op=ALU.mult)
                        PQv = GQ[:].rearrange("p (m s) two -> p m s two", m=QC)
                        for c8 in range(QC):
                            for slot in range(2):
                                nc.tensor.matmul(
                                    psums[c8][:], lhsT=WK2[:, tap, :],
                                    rhs=PQv[:, c8, :, slot],
                                    start=(tap == 0 and slot == 0),
                                    stop=(tap == T - 1 and slot == 1))
                    OSBQ = gp.tile([C, QC, CHUNK], f32, tag="OSBQ")
                    for c8 in range(QC):
                        nc.scalar.copy(out=OSBQ[:, c8, :], in_=psums[c8][:])
                    nc.sync.dma_start(
                        out=out_dram.ap()[:, Q * QC * CHUNK:(Q + 1) * QC * CHUNK],
                        in_=OSBQ[:].rearrange("p a b -> p (a b)"))

    nc.compile()
    return nc


def _prep_weights(w_offset, b_offset, w_conv):
    w_offset = np.asarray(w_offset, dtype=np.float32)
    w_conv = np.asarray(w_conv, dtype=np.float32)
    b_offset = np.asarray(b_offset, dtype=np.float32)
    # woffT[c, t*27 + j] = w_offset[j, c, ty, tx]
    woffT = w_offset.transpose(2, 3, 1, 0).reshape(T, C, NJ)  # [t, c, j]
    woffT = woffT.transpose(1, 0, 2).reshape(C, T * NJ).copy()
    boff = b_offset.reshape(NJ, 1).copy()
    # wk2[q, t*64 + o] = w_conv[o, q%64, ty, tx]
    wkt = w_conv.transpose(2, 3, 1, 0).reshape(T, C, C)       # [t, c, o]
    wk2 = np.concatenate([wkt, wkt], axis=1)                   # [t, 128, o]
    wk2 = wk2.transpose(1, 0, 2).reshape(128, T * C).astype(np_bf16).copy()
    return woffT, boff, wk2


def kernel(x, w_offset, b_offset, w_conv):
    x = np.asarray(x, dtype=np.float32)
    woffT, boff, wk2 = _prep_weights(w_offset, b_offset, w_conv)
    if "nc" not in _CACHE:
        _CACHE["nc"] = build_nc(num_devices=B)
    nc = _CACHE["nc"]
    in_maps = []
    for b in range(B):
        in_maps.append({
            "x_in": np.ascontiguousarray(x[b].reshape(C, HW)),
            "woffT_in": woffT,
            "boff_in": boff,
            "wk2_in": wk2,
        })
    res = run_bass_kernel_spmd(nc, in_maps, core_ids=list(range(B)))
    out = np.stack([res.results[b]["out"].reshape(C, H, W) for b in range(B)])
    return out.astype(np.float32)


# revision 41
# speedup vs baseline: 1.2372x; 1.2372x over previous
"""Deformable conv (DCNv2-style) TRN2 Bass kernel — DMA-restructured.

Problem: x[8,64,128,128] f32; offset conv (27ch 3x3) -> (dy,dx,mask) per 9 taps;
bilinear sampling of x at tap positions + offsets; modulated; 3x3 conv via
per-tap 1x1 matmuls.  Data-parallel over batch, 1 sample per core.

Per-core pipeline:
  A) om conv: 9 shifted f32r matmuls per 512-pos chunk on zero-padded XP
     -> OM_ALL [27, 16384]; ONE transpose DMA -> OMT [h, j, w].
  B) stage2 (DVE/ACT on [128h, 9t, 128w]): floor/clip/hat-weights/sigmoid
     -> W4C (bf16 corner-weight pairs) and IDXF (u16 element idx into X3).
  C) X3: interleaved row-pair layout X3[c, 2*(128r+j)+s] = x[c, r+s, j] (bf16);
     partitions 64-127 hold X3 shifted by 2 elements (x0+1 corners).
  D) IDXT: 16+3 DMAs -> idx wrapped per 16 partitions [p, t, m, col].
     WLIN staging: W4C -> DRAM as [t][half, m, hl, w, slot] (18 small DMAs).
  E) main loop over 4 Qs x 9 taps: ONE indirect_copy (4096 idx, d=2),
     ONE 2 MB 128-partition broadcast DMA for weights, ONE DVE mult,
     16 accumulating matmuls -> PSUM [64,512] x 8 chunks; per Q: 8 ACT
     copies -> OSBQ, ONE 1 MB DMA to out.

Gather stream order: position s = 512*m + col*16 + p.
"""
import numpy as np
import ml_dtypes

from concourse.bacc import Bacc
from concourse import mybir, tile
from concourse.bass_utils import run_bass_kernel_spmd

np_bf16 = np.dtype(ml_dtypes.bfloat16)
f32 = mybir.dt.float32
f32r = mybir.dt.float32r
bf16 = mybir.dt.bfloat16
u16 = mybir.dt.uint16
i32 = mybir.dt.int32

B, C, H, W = 8, 64, 128, 128
HW = H * W          # 16384
T = 9               # taps
NJ = 27             # offset-conv channels
NCHUNK = 32         # 512-position chunks
CHUNK = 512
NQ = 4              # chunk groups
QC = NCHUNK // NQ   # 8 chunks per group
AF = mybir.ActivationFunctionType
ALU = mybir.AluOpType

_CACHE = {}


def _host_consts():
    # CYK[h, t] = h + ky(t) - 1 ; CXW[h, t, w] = w + kx(t) - 1 (h-independent)
    ky = np.arange(T) // 3
    kx = np.arange(T) % 3
    cyk = (np.arange(128)[:, None] + ky[None, :] - 1).astype(np.float32)
    cxw = np.broadcast_to(
        (np.arange(128)[None, :] + kx[:, None] - 1)[None, :, :], (128, T, 128)
    ).astype(np.float32).copy()
    return cyk, cxw


def build_nc(num_devices=8):
    import os
    variant = os.environ.get("KVARIANT", "full")
    v_wrq2 = "wrq2" in variant or "oldall" in variant
    v_omtold = "omtold" in variant or "oldall" in variant
    v_idxold = "idxold" in variant or "oldall" in variant
    nc = Bacc("TRN2", target_bir_lowering=False, debug=False,
              num_devices=num_devices)
    kdebug = os.environ.get("KDEBUG") == "1"

    x_in = nc.dram_tensor("x_in", [C, HW], f32, kind="ExternalInput")
    woffT_in = nc.dram_tensor("woffT_in", [C, T * NJ], f32, kind="ExternalInput")
    boff_in = nc.dram_tensor("boff_in", [NJ, 1], f32, kind="ExternalInput")
    wk2_in = nc.dram_tensor("wk2_in", [128, T * C], bf16, kind="ExternalInput")
    out_dram = nc.dram_tensor("out", [C, HW], f32, kind="ExternalOutput")

    cyk_np, cxw_np = _host_consts()
    cyk_const = nc.inline_tensor(cyk_np, name="cyk_const")
    cxw_const = nc.inline_tensor(cxw_np.reshape(128, T * 128), name="cxw_const")

    with tile.TileContext(nc) as tc:
        with tc.tile_pool(name="main", bufs=1) as mp, \
             tc.tile_pool(name="dram", bufs=1, space="DRAM") as drp:
            # ---------- persistent tiles ----------
            X3 = mp.tile([128, HW * 2 + 16], bf16, tag="X3")     # 64 KiB + pad
            W4C = mp.tile([128, 2, T, 128, 2], bf16, tag="W4C")  # [h,half,t,w,slot]
            IDXF = mp.tile([128, T, 8, 16], u16, tag="IDXF")     # [h,t,a,b] (a=w//16,b=w%16)
            IDXTQ = [mp.tile([128, T, QC, 32], u16, tag=f"IDXT{q}",
                             name=f"IDXT_{q}") for q in range(NQ)]
            # WLIN row (t*2+half) = [m, hl, w, slot] flat; DRAM staging for
            # broadcast reads (flat 2-D: >2-D DRAM tile slicing miscompiles)
            WLIN = drp.tile([T * 2, NCHUNK * 4 * 128 * 2], bf16, tag="WLIN")
            CYK = mp.tile([128, T], f32, tag="CYK")
            CXW = mp.tile([128, T, 128], f32, tag="CXW")
            WOFFT = mp.tile([C, T, NJ], bf16, tag="WOFFT")
            WK2 = mp.tile([128, T, C], bf16, tag="WK2")
            BOFF = mp.tile([NJ, 1], f32, tag="BOFF")

            nc.sync.dma_start(CYK[:], cyk_const.ap())
            nc.sync.dma_start(CXW[:].rearrange("p a b -> p (a b)"), cxw_const.ap())
            nc.sync.dma_start(WK2[:].rearrange("p a b -> p (a b)"), wk2_in.ap())
            nc.sync.dma_start(BOFF[:], boff_in.ap())

            # ================= Phase A: pad + X3 + om conv =================
            midcm = tc.tile_pool(name="mid", bufs=1)
            midp = midcm.__enter__()
            OMT = midp.tile([128, NJ, 128], f32, tag="OMT")      # [h, j, w]
            with tc.tile_pool(name="early", bufs=1) as ep, \
                 tc.tile_pool(name="omdb", bufs=2) as odb, \
                 tc.tile_pool(name="ompsum", bufs=2, space="PSUM") as opp:
                XP = ep.tile([C, 130 * 130], bf16, tag="XP")
                WOFFS = ep.tile([C, T * NJ], f32, tag="WOFFS")
                nc.sync.dma_start(WOFFS[:], woffT_in.ap())
                nc.vector.tensor_copy(out=WOFFT[:].rearrange("p a b -> p (a b)"),
                                      in_=WOFFS[:])

                # zero only the halo ring of XP (interior is overwritten by x)
                XP3 = XP[:].rearrange("p (r c2) -> p r c2", c2=130)
                nc.vector.memset(XP[:, 0:130], 0.0)                  # top row
                nc.vector.memset(XP[:, 129 * 130:130 * 130], 0.0)    # bottom row
                nc.vector.memset(XP3[:, 1:129, 0], 0.0)              # left col
                nc.vector.memset(XP3[:, 1:129, 129], 0.0)            # right col
                nc.gpsimd.dma_start(out=XP3[:, 1:129, 1:129], in_=x_in.ap())

                # X3 A-half: X3[c, r*256 + 2j + s] = x[c, r+s, j]
                X3A = X3[0:64, 0:HW * 2].rearrange("p (r j s) -> p r j s", j=128, s=2)
                nc.scalar.copy(out=X3A[:, :, :, 0], in_=XP3[0:64, 1:129, 1:129])
                nc.vector.tensor_copy(out=X3A[:, :, :, 1], in_=XP3[0:64, 2:130, 1:129])
                nc.vector.memset(X3[0:64, HW * 2:], 0.0)
                # B-half: shift by 2 elements (cross-partition copy via DMA)
                nc.sync.dma_start(out=X3[64:128, 0:2 * HW - 2], in_=X3[0:64, 2:2 * HW])
                nc.vector.memset(X3[64:128, 2 * HW - 2:], 0.0)

                # om conv: per 512-pos chunk, 9 accumulating f32r matmuls;
                # stage 4 chunks into omp4 -> DRAM scratch -> transpose load
                OMD = drp.tile([NJ, HW], f32, tag="OMD")   # DRAM [j, (h w)]
                for g in range(NCHUNK // 4):
                    omp4 = odb.tile([NJ, 4, CHUNK], f32, tag="omp4")
                    for mi in range(4):
                        m = 4 * g + mi
                        ps = opp.tile([NJ, CHUNK], f32, tag="omps")
                        for t9 in range(T):
                            ty, tx = divmod(t9, 3)
                            rhs = XP3[0:64, 4 * m + ty: 4 * m + ty + 4, tx: tx + 128]
                            nc.tensor.matmul(ps[:], lhsT=WOFFT[:, t9, :], rhs=rhs,
                                             start=(t9 == 0), stop=(t9 == T - 1))
                        nc.scalar.activation(out=omp4[:, mi, :], in_=ps[:],
                                             func=AF.Identity, bias=BOFF[:])
                    if v_omtold:
                        ompv = omp4[:].rearrange("j m4 (hp w) -> j (m4 hp) w", w=128)
                        for hp in range(16):
                            nc.sync.dma_start(
                                out=OMT[16 * g + hp:16 * g + hp + 1, :, :],
                                in_=ompv[:, hp, :])
                    else:
                        nc.sync.dma_start(
                            out=OMD[:][:, 2048 * g:2048 * (g + 1)],
                            in_=omp4[:].rearrange("j m4 p -> j (m4 p)"))
                if not v_omtold:
                    # ONE transpose load: OMT[h, j, w] <- OMD[j, (h w)]
                    nc.sync.dma_start(
                        out=OMT[:],
                        in_=OMD[:].rearrange("j (h w) -> h j w", w=128))
            # ================= stage 2: weights + idx =================
            with tc.tile_pool(name="s2", bufs=1) as sp:
                OMTv = OMT[:]
                DY = OMTv[:, 0:18, :].rearrange("p (k s) w -> p k s w", s=2)[:, :, 0, :]
                DX = OMTv[:, 0:18, :].rearrange("p (k s) w -> p k s w", s=2)[:, :, 1, :]
                MS = OMTv[:, 18:27, :]

                sh = [128, T, 128]
                YS = sp.tile(sh, f32, tag="YS")
                XS = sp.tile(sh, f32, tag="XS")
                Y0C = sp.tile(sh, f32, tag="Y0C")
                X0C = sp.tile(sh, f32, tag="X0C")
                TMPI = sp.tile(sh, i32, tag="TMPI")
                TY = sp.tile(sh, f32, tag="TY")
                TX = sp.tile(sh, f32, tag="TX")
                WYA = sp.tile(sh, f32, tag="WYA")
                WYB = sp.tile(sh, f32, tag="WYB")
                WXA = sp.tile(sh, f32, tag="WXA")
                WXB = sp.tile(sh, f32, tag="WXB")
                MSK = sp.tile(sh, f32, tag="MSK")
                TMP = sp.tile(sh, f32, tag="TMP")
                TMP2 = sp.tile(sh, f32, tag="TMP2")

                CYKb = CYK[:].unsqueeze(2).broadcast_to(sh)

                # ys/xs
                nc.vector.tensor_tensor(out=YS[:], in0=DY, in1=CYKb, op=ALU.add)
                nc.vector.tensor_tensor(out=XS[:], in0=DX, in1=CXW[:], op=ALU.add)
                # floor via round(x - 0.5) (cast round-to-nearest-even)
                for SRC, DSTF in ((YS, Y0C), (XS, X0C)):
                    nc.vector.tensor_scalar(out=TMP[:], in0=SRC[:], scalar1=0.5,
                                            scalar2=None, op0=ALU.subtract)
                    nc.vector.tensor_copy(out=TMPI[:], in_=TMP[:])
                    nc.vector.tensor_copy(out=DSTF[:], in_=TMPI[:])
                    # clip to [0, 127]
                    nc.vector.tensor_scalar(out=DSTF[:], in0=DSTF[:], scalar1=0.0,
                                            scalar2=127.0, op0=ALU.max, op1=ALU.min)
                # t = s - clip ; weights
                nc.vector.tensor_tensor(out=TY[:], in0=YS[:], in1=Y0C[:], op=ALU.subtract)
                nc.vector.tensor_tensor(out=TX[:], in0=XS[:], in1=X0C[:], op=ALU.subtract)
                # wA = relu(1 - |t|), wBr = relu(t)
                nc.scalar.activation(out=TMP[:], in_=TY[:], func=AF.Abs)
                nc.scalar.activation(out=WYA[:], in_=TMP[:], func=AF.Relu, scale=-1.0, bias=1.0)
                nc.scalar.activation(out=WYB[:], in_=TY[:], func=AF.Relu)
                nc.scalar.activation(out=TMP2[:], in_=TX[:], func=AF.Abs)
                nc.scalar.activation(out=WXA[:], in_=TMP2[:], func=AF.Relu, scale=-1.0, bias=1.0)
                nc.scalar.activation(out=WXB[:], in_=TX[:], func=AF.Relu)
                # upper-boundary masks: wyB *= (ys < 127); wxB *= (xs < 127)
                nc.vector.tensor_scalar(out=TMP[:], in0=YS[:], scalar1=127.0,
                                        scalar2=None, op0=ALU.is_lt)
                nc.vector.tensor_tensor(out=WYB[:], in0=WYB[:], in1=TMP[:], op=ALU.mult)
                nc.vector.tensor_scalar(out=TMP2[:], in0=XS[:], scalar1=127.0,
                                        scalar2=None, op0=ALU.is_lt)
                nc.vector.tensor_tensor(out=WXB[:], in0=WXB[:], in1=TMP2[:], op=ALU.mult)
                # mask; fold into wx
                nc.scalar.activation(out=MSK[:], in_=MS, func=AF.Sigmoid)
                nc.vector.tensor_tensor(out=WXA[:], in0=WXA[:], in1=MSK[:], op=ALU.mult)
                nc.vector.tensor_tensor(out=WXB[:], in0=WXB[:], in1=MSK[:], op=ALU.mult)
                # products -> W4C (bf16, interleaved)
                nc.vector.tensor_tensor(out=W4C[:, 0, :, :, 0], in0=WYA[:], in1=WXA[:], op=ALU.mult)
                nc.vector.tensor_tensor(out=W4C[:, 0, :, :, 1], in0=WYB[:], in1=WXA[:], op=ALU.mult)
                nc.vector.tensor_tensor(out=W4C[:, 1, :, :, 0], in0=WYA[:], in1=WXB[:], op=ALU.mult)
                nc.vector.tensor_tensor(out=W4C[:, 1, :, :, 1], in0=WYB[:], in1=WXB[:], op=ALU.mult)
                # idx (element units into X3): 2*(y0c*128 + x0c), written in
                # IDXF layout [h, t, a, b] with value for w = 16a + b
                nc.vector.scalar_tensor_tensor(
                    out=TMP[:], in0=Y0C[:], scalar=128.0, in1=X0C[:],
                    op0=ALU.mult, op1=ALU.add)
                IDXF_w = IDXF[:].rearrange("p t a b2 -> p (t a b2)").rearrange(
                    "p (t a b2) -> p t a b2", t=T, a=8)
                nc.vector.tensor_scalar(
                    out=IDXF_w, in0=TMP[:].rearrange("p t (a b2) -> p t a b2", a=8),
                    scalar1=2.0, scalar2=None, op0=ALU.mult)

            # ---------- IDXT build: DRAM bounce + 16 loads + 3 replications ----
            # IDXT[p, t, m, col] = idx(s = 512m + col*16 + p)
            #                    = IDXF[h = 4m + col//8, t, a = col%8, b = p]
            # hardware-xbar transpose: ITF[(a b), t, h] = IDXF[h, t, a, b]
            ITF = midp.tile([128, T, 128], u16, tag="ITF")
            for t9 in range(T):
                nc.sync.dma_start_transpose(
                    out=ITF[:, t9, :],
                    in_=IDXF[:, t9, :, :].rearrange("p a b -> p (a b)"))
            # bounce through DRAM:
            # IDXT[p, t, m, (hl a)] = ITF[16a+p, t, (m hl)]
            IDXD = drp.tile([128, T * 128], u16, tag="IDXD")
            nc.sync.dma_start(out=IDXD[:],
                              in_=ITF[:].rearrange("p t h -> p (t h)"))
            # per (Q, a) loads into per-Q tiles; later Qs overlap main loop
            IDXD4 = IDXD[:].rearrange("p (t m hl) -> p t m hl",
                                      m=NCHUNK, hl=4)
            for Qq in range(NQ):
                for a8 in range(8):
                    nc.sync.dma_start(
                        out=IDXTQ[Qq][0:16, :, :, a8::8].rearrange(
                            "p t m hl -> p (t m hl)"),
                        in_=IDXD4[16 * a8:16 * (a8 + 1), :,
                                  QC * Qq:QC * (Qq + 1), :]
                            .rearrange("p t m hl -> p t (m hl)"))
                # replicate partitions 0-15 -> 16-127 (doubling)
                qs = IDXTQ[Qq]
                nc.sync.dma_start(out=qs[16:32], in_=qs[0:16])
                nc.sync.dma_start(out=qs[32:64], in_=qs[0:32])
                nc.sync.dma_start(out=qs[64:128], in_=qs[0:64])
            midcm.__exit__(None, None, None)

            if kdebug:
                d_omt = nc.dram_tensor("d_omt", [128, NJ * 128], f32, kind="ExternalOutput")
                nc.sync.dma_start(d_omt.ap(), OMT[:].rearrange("p a b -> p (a b)"))
                d_w4 = nc.dram_tensor("d_w4", [128, 2 * T * 128 * 2], bf16, kind="ExternalOutput")
                nc.sync.dma_start(d_w4.ap(), W4C[:].rearrange("p a b c d -> p (a b c d)"))
                d_idxt = nc.dram_tensor("d_idxt", [128, T * NCHUNK * 32], u16, kind="ExternalOutput")
                d_idxt4 = d_idxt.ap().rearrange("p (t m c) -> p t m c", t=T, m=NCHUNK)
                for q in range(NQ):
                    nc.sync.dma_start(
                        d_idxt4[:, :, QC * q:QC * (q + 1), :]
                            .rearrange("p t m c -> p t (m c)"),
                        IDXTQ[q][:].rearrange("p t m c -> p t (m c)"))
                d_x3 = nc.dram_tensor("d_x3", [128, HW * 2], bf16, kind="ExternalOutput")
                nc.sync.dma_start(d_x3.ap(), X3[:, 0:HW * 2])

            # ---------- WLIN build: 18 small DMAs (SBUF -> DRAM) ----------
            # WLIN[t*2+half] = flat [m, hl, w, slot] = W4C[h=(m hl), half, t, w, s]
            WLINv = WLIN[:]
            for t9 in range(T):
                for half in range(2):
                    nc.sync.dma_start(
                        out=WLINv[t9 * 2 + half].rearrange("(h f) -> h f", h=128),
                        in_=W4C[:, half, t9, :, :].rearrange("p a b -> p (a b)"))

            if kdebug:
                d_wlin = nc.dram_tensor("d_wlin", [T * 2, NCHUNK * 4 * 128 * 2], bf16, kind="ExternalOutput")
                nc.sync.dma_start(d_wlin.ap(), WLINv)

            # ================= main loop =================
            with tc.tile_pool(name="gl", bufs=2) as gp, \
                 tc.tile_pool(name="ps2", bufs=1, space="PSUM") as pp2:
                X3v = X3[:, 0:HW * 2].rearrange("p (n s) -> p n s", s=2)
                # absorb initial deps into gpsimd queue
                dd1 = mp.tile([128, 1], bf16, tag="dd1")
                dd2 = mp.tile([128, 1], u16, tag="dd2")
                nc.gpsimd.tensor_copy(out=dd1[:], in_=X3[:, 0:1])
                nc.gpsimd.tensor_copy(out=dd2[:], in_=IDXTQ[0][:, 0, 0, 0:1])

                for Q in range(NQ):
                    psums = [pp2.tile([C, CHUNK], f32, tag=f"eps{c8}",
                                      name=f"eps_{Q}_{c8}")
                             for c8 in range(QC)]
                    for tap in range(T):
                        GQ = gp.tile([128, QC * CHUNK, 2], bf16, tag="GQ")
                        WRQ = gp.tile([128, QC, CHUNK, 2], bf16, tag="WRQ")
                        # gathers: per chunk (IC dst cap = 1024 elems/partition)
                        for c8 in range(QC):
                            nc.gpsimd.indirect_copy(
                                out=GQ[:, CHUNK * c8:CHUNK * (c8 + 1), :],
                                data=X3v,
                                idxs=IDXTQ[Q][:, tap, c8, :],
                                i_know_ap_gather_is_preferred=True)
                        # 128-partition broadcast DMA (2 MB):
                        # WRQ[(half c), m, pos, slot] <- WLIN[tap, half, m(Q), :, :, :]
                        QB = QC * CHUNK * 2   # elems per (tap, half, Q)
                        WRQf = WRQ[:].rearrange("p m pos s -> p (m pos s)")
                        for half in range(2):
                            eng = [nc.scalar, nc.sync,
                                   nc.gpsimd][(2 * tap + half) % 3]
                            eng.dma_start(
                                out=WRQf[64 * half:64 * (half + 1)],
                                in_=WLINv[tap * 2 + half,
                                          QB * Q:QB * (Q + 1)]
                                    .unsqueeze(0)
                                    .broadcast_to([64, QB]))
                        if kdebug and Q == 0 and tap == 0:
                            d_gq = nc.dram_tensor("d_gq", [128, QC * CHUNK * 2], bf16, kind="ExternalOutput")
                            nc.sync.dma_start(d_gq.ap(), GQ[:].rearrange("p a b -> p (a b)"))
                            d_wrq = nc.dram_tensor("d_wrq", [128, QC * CHUNK * 2], bf16, kind="ExternalOutput")
                            nc.sync.dma_start(d_wrq.ap(), WRQ[:].rearrange("p a b c -> p (a b c)"))
                        nc.vector.tensor_tensor(
                            out=GQ[:].rearrange("p a b -> p (a b)"),
                            in0=GQ[:].rearrange("p a b -> p (a b)"),
                            in1=WRQ[:].rearrange("p a b c -> p (a b c)"),
                            op=ALU.mult)
                        PQv = GQ[:].rearrange("p (m s) two -> p m s two", m=QC)
                        for c8 in range(QC):
                            for slot in range(2):
                                nc.tensor.matmul(
                                    psums[c8][:], lhsT=WK2[:, tap, :],
                                    rhs=PQv[:, c8, :, slot],
                                    start=(tap == 0 and slot == 0),
                                    stop=(tap == T - 1 and slot == 1))
                    OSBQ = gp.tile([C, QC, CHUNK], f32, tag="OSBQ")
                    for c8 in range(QC):
                        nc.scalar.copy(out=OSBQ[:, c8, :], in_=psums[c8][:])
                    nc.sync.dma_start(
                        out=out_dram.ap()[:, Q * QC * CHUNK:(Q + 1) * QC * CHUNK],
                        in_=OSBQ[:].rearrange("p a b -> p (a b)"))

    nc.compile()
    return nc


def _prep_weights(w_offset, b_offset, w_conv):
    w_offset = np.asarray(w_offset, dtype=np.float32)
    w_conv = np.asarray(w_conv, dtype=np.float32)
    b_offset = np.asarray(b_offset, dtype=np.float32)
    # woffT[c, t*27 + j] = w_offset[j, c, ty, tx]
    woffT = w_offset.transpose(2, 3, 1, 0).reshape(T, C, NJ)  # [t, c, j]
    woffT = woffT.transpose(1, 0, 2).reshape(C, T * NJ).copy()
    boff = b_offset.reshape(NJ, 1).copy()
    # wk2[q, t*64 + o] = w_conv[o, q%64, ty, tx]
    wkt = w_conv.transpose(2, 3, 1, 0).reshape(T, C, C)       # [t, c, o]
    wk2 = np.concatenate([wkt, wkt], axis=1)                   # [t, 128, o]
    wk2 = wk2.transpose(1, 0, 2).reshape(128, T * C).astype(np_bf16).copy()
    return woffT, boff, wk2


def kernel(x, w_offset, b_offset, w_conv):
    x = np.asarray(x, dtype=np.float32)
    woffT, boff, wk2 = _prep_weights(w_offset, b_offset, w_conv)
    if "nc" not in _CACHE:
        _CACHE["nc"] = build_nc(num_devices=B)
    nc = _CACHE["nc"]
    in_maps = []
    for b in range(B):
        in_maps.append({
            "x_in": np.ascontiguousarray(x[b].reshape(C, HW)),
            "woffT_in": woffT,
            "boff_in": boff,
            "wk2_in": wk2,
        })
    res = run_bass_kernel_spmd(nc, in_maps, core_ids=list(range(B)))
    out = np.stack([res.results[b]["out"].reshape(C, H, W) for b in range(B)])
    return out.astype(np.float32)
